# revision 6
# baseline (speedup 1.0000x reference)
"""ESM2 contact predictor head on 8 Trainium2 NeuronCores.

Computes out[b, i, j] = sigmoid(x[b,i] @ W @ x[b,j] + bias) for
x: (8, 2050, 320) f32, W: (320, 320) f32, bias: (1,) f32.

Sharding: data-parallel over batch — core c handles batch element c.

Per-core algorithm (all matmuls in bf16 with f32 PSUM accumulation;
bf16 streams at the full PE rate and halves DMA bytes, weight-load
time and SBUF footprint vs f32r; total quantization error ~5e-3 rel,
inside the 2e-2 gate):
  host:  xt = bf16(x[c]).T as 3 K-slabs of 128 partitions (D=320
         zero-padded to 384), pre-chunked so input DMAs are contiguous;
         wp = bf16(W) zero-padded to (384, 384).
  chip:  warmup matmuls release the PE clock-gate while inputs stream in;
         u = wp.T @ xt                      == (x[c] @ W).T, cast to bf16
         (phase 1 runs its chunk-0 halves first so the PE never waits on
         the chunk-1 input DMA);
         out[i, j] = sigmoid(sum_e u[e,i] * xt[e,j] + bias), produced as
         128-row strips: per 1024-col half, 6 matmuls (k-outer, shared
         stationary operand, alternating PSUM banks) -> fused sigmoid+bias
         on ScalarE reading PSUM into a [128, 2048] bf16 strip tile; one
         contiguous 512KB DMA per strip.
         The 2 tail columns (j=2048:2050) are computed transposed
         (2 partitions x 2050) into a separate DRAM tensor; the host
         transposes them into place. Tail work runs mid-schedule; the
         last strip drains as a half plus two 512-col quarters whose
         first sigmoid overlaps the remaining matmuls.
"""

import numpy as np
import ml_dtypes

import concourse.mybir as mybir
import concourse.tile as tile
from concourse import bacc
from concourse.bass_utils import run_bass_kernel_spmd

N_CORES = 8
B, L, D = 8, 2050, 320
KT = 3            # K slabs: 128, 128, 64(zero-padded)
F32 = mybir.dt.float32
BF16 = mybir.dt.bfloat16
SIG = mybir.ActivationFunctionType.Sigmoid

J_TAIL = 2048
CHUNK = 512       # input DMA chunk (columns)

_cache = {}


def _build(bias_val: float):
    nc = bacc.Bacc("TRN2", target_bir_lowering=False, debug=False,
                   num_devices=N_CORES)
    xt_main_d = nc.dram_tensor("xt_main", [4, 128, KT, CHUNK], BF16,
                               kind="ExternalInput")
    xt_tail_d = nc.dram_tensor("xt_tail", [128, KT, 2], BF16,
                               kind="ExternalInput")
    w_d = nc.dram_tensor("w", [384, 384], BF16, kind="ExternalInput")
    out_d = nc.dram_tensor("out", [L, J_TAIL], BF16, kind="ExternalOutput")
    outt_d = nc.dram_tensor("out_tail_t", [2, L], BF16, kind="ExternalOutput")

    w_r = w_d.ap().rearrange("(k p) e -> p k e", p=128)     # (128, 3, 384)

    with tile.TileContext(nc) as tc:
        with (
            tc.tile_pool(name="persist", bufs=1) as pp,
            tc.tile_pool(name="outp", bufs=6) as outp,
            tc.tile_pool(name="psum", bufs=2, space="PSUM") as psp,
        ):
            bias_t = pp.tile([128, 1], F32)
            nc.vector.memset(bias_t[:], bias_val)

            w_sb = pp.tile([128, KT, 384], BF16)
            xt_sb = pp.tile([128, KT, L], BF16)
            nc.sync.dma_start(w_sb[:], w_r[:])
            nc.sync.dma_start(xt_sb[:, :, 0:CHUNK], xt_main_d.ap()[0])
            nc.sync.dma_start(xt_sb[:, :, CHUNK:2 * CHUNK], xt_main_d.ap()[1])
            nc.sync.dma_start(xt_sb[:, :, 2 * CHUNK:3 * CHUNK], xt_main_d.ap()[2])
            nc.sync.dma_start(xt_sb[:, :, 3 * CHUNK:4 * CHUNK], xt_main_d.ap()[3])
            nc.sync.dma_start(xt_sb[:, :, J_TAIL:L], xt_tail_d.ap())

            u_sb = pp.tile([128, KT, L], BF16)

            # PE warmup: dummy matmuls during the input-DMA window so the
            # HAM clock-gate is released before real work starts; the ramp
            # needs ~3us of continuous execution.
            warm_sb = pp.tile([128, 256], F32)
            nc.vector.memset(warm_sb[:], 1.0)
            warm16 = warm_sb.bitcast(BF16)           # (128, 512)
            psw = psp.tile([128, 1024], F32, tag="pair", bufs=3, name="psw")
            for wi in range(7):
                h = (wi % 2) * 512
                nc.tensor.matmul(psw[:, h:h + 512], lhsT=warm16[:, :128],
                                 rhs=warm16[:], start=True, stop=True)
            # preload the sigmoid activation table while DMAs run
            act_warm = pp.tile([128, 1], F32)
            nc.scalar.activation(act_warm[:], bias_t[:], SIG)

            def mm_acc(ps, lhs, j0, nsz):
                """One accumulation group over the 3 K slabs (slab 2
                zero-padded to K=128)."""
                for k in range(KT):
                    nc.tensor.matmul(ps, lhsT=lhs(k),
                                     rhs=xt_sb[:, k, j0:j0 + nsz],
                                     start=(k == 0), stop=(k == KT - 1))

            # ---- phase 1: u = wp.T @ xt  (u[e, i], e on partitions) ----
            def wlhs_of(et):
                def wlhs(k, e0=et * 128):
                    return w_sb[:, k, e0:e0 + 128]
                return wlhs

            def ph1_ntp0():
                # chunk-0 halves of all three e-strips first, so the PE has
                # ~2us of work that doesn't depend on the chunk-1 DMA.
                tiles = []
                for et in range(KT):
                    ps1 = psp.tile([128, 1024], F32, tag="pair", bufs=3,
                                   name="ps1")
                    mm_acc(ps1[:, 0:512], wlhs_of(et), 0, 512)
                    tiles.append(ps1)
                for et in range(KT):
                    ps1 = tiles[et]
                    mm_acc(ps1[:, 512:1024], wlhs_of(et), 512, 512)
                    nc.vector.tensor_copy(u_sb[:, et, 0:1024], ps1[:, :])

            def ph1_ntp1():
                n0 = 1024
                for et in range(KT):
                    ps1 = psp.tile([128, 1024], F32, tag="pair", bufs=3,
                                   name="ps1")
                    mm_acc(ps1[:, 0:512], wlhs_of(et), n0, 512)
                    mm_acc(ps1[:, 512:1024], wlhs_of(et), n0 + 512, 512)
                    nc.vector.tensor_copy(u_sb[:, et, n0:n0 + 1024], ps1[:, :])

            def ph1_tail():
                # u[:, 2048:2050] for the three e-strips in one psum tile
                # (three 2-col accumulation groups at offsets 0/2/4).
                psT = psp.tile([128, 512], F32, tag="small", bufs=2,
                               name="psT")
                for et in range(KT):
                    mm_acc(psT[:, 2 * et:2 * et + 2], wlhs_of(et), J_TAIL, 2)
                for et in range(KT):
                    nc.vector.tensor_copy(u_sb[:, et, J_TAIL:L],
                                          psT[:, 2 * et:2 * et + 2])

            def tail_block():
                # out[:, 2048:2050] for all i, computed transposed (j on
                # partitions), written to its own DRAM tensor; host
                # transposes it into place. Two pair psum tiles hold the
                # four 512-col accumulation groups.
                outT = outp.tile([2, L], BF16, tag="ttail", bufs=1, name="outT")
                for half in range(2):
                    psc = psp.tile([128, 1024], F32, tag="pair", bufs=3,
                                   name="psc")
                    for q in range(2):
                        c0 = (2 * half + q) * 512
                        for k in range(KT):
                            nc.tensor.matmul(psc[:2, q * 512:(q + 1) * 512],
                                             lhsT=xt_sb[:, k, J_TAIL:L],
                                             rhs=u_sb[:, k, c0:c0 + 512],
                                             start=(k == 0), stop=(k == KT - 1))
                    for q in range(2):
                        c0 = (2 * half + q) * 512
                        nc.scalar.activation(outT[:, c0:c0 + 512],
                                             psc[:2, q * 512:(q + 1) * 512],
                                             SIG, bias=bias_t[:2, :])
                psc2 = psp.tile([128, 512], F32, tag="small", bufs=2,
                                name="psc2")
                for k in range(KT):
                    nc.tensor.matmul(psc2[:2, :2], lhsT=xt_sb[:, k, J_TAIL:L],
                                     rhs=u_sb[:, k, J_TAIL:L],
                                     start=(k == 0), stop=(k == KT - 1))
                nc.scalar.activation(outT[:, J_TAIL:L], psc2[:2, :2], SIG,
                                     bias=bias_t[:2, :])
                nc.sync.dma_start(outt_d.ap()[:], outT[:])

            def half_mms(ps, i0, isz, jp):
                a0 = jp * 1024
                for k in range(KT):
                    u_k = u_sb[:, k, i0:i0 + isz]
                    nc.tensor.matmul(ps[:isz, 0:512], lhsT=u_k,
                                     rhs=xt_sb[:, k, a0:a0 + 512],
                                     start=(k == 0), stop=(k == KT - 1))
                    nc.tensor.matmul(ps[:isz, 512:1024], lhsT=u_k,
                                     rhs=xt_sb[:, k, a0 + 512:a0 + 1024],
                                     start=(k == 0), stop=(k == KT - 1))

            def do_strip(i0):
                # full 2048-col strip: two halves -> two sigmoids into one
                # [128, 2048] tile -> one contiguous 512KB DMA.
                big = outp.tile([128, 2048], BF16, tag="strip", bufs=6,
                                name="big")
                for jp in range(2):
                    ps = psp.tile([128, 1024], F32, tag="pair", bufs=3,
                                  name="ps")
                    half_mms(ps, i0, 128, jp)
                    nc.scalar.activation(big[:, jp * 1024:(jp + 1) * 1024],
                                         ps[:, :], SIG, bias=bias_t[:, :])
                nc.sync.dma_start(out_d.ap()[i0:i0 + 128, :], big[:, :])

            def do_half(i0, isz, jp):
                # one 1024-col half with its own DMA (i-tail, final strip)
                a0 = jp * 1024
                ps = psp.tile([128, 1024], F32, tag="pair", bufs=3, name="ps")
                half_mms(ps, i0, isz, jp)
                half = outp.tile([128, 1024], BF16, tag="half", bufs=2,
                                 name="half")
                nc.scalar.activation(half[:isz, :], ps[:isz, :], SIG,
                                     bias=bias_t[:isz, :])
                nc.sync.dma_start(out_d.ap()[i0:i0 + isz, a0:a0 + 1024],
                                  half[:isz, :])

            def do_final(i0):
                # last half (jp=1): quarter-grouped matmuls so the first
                # sigmoid overlaps the remaining three matmuls; per-quarter
                # DMA for a fast drain.
                a0 = 1024
                ps = psp.tile([128, 1024], F32, tag="pair", bufs=3, name="ps")
                for q in range(2):
                    for k in range(KT):
                        nc.tensor.matmul(
                            ps[:, q * 512:(q + 1) * 512],
                            lhsT=u_sb[:, k, i0:i0 + 128],
                            rhs=xt_sb[:, k, a0 + q * 512:a0 + (q + 1) * 512],
                            start=(k == 0), stop=(k == KT - 1))
                for q in range(2):
                    qt = outp.tile([128, 512], BF16, tag="fin", bufs=2,
                                   name="fin")
                    nc.scalar.activation(qt[:, :],
                                         ps[:, q * 512:(q + 1) * 512],
                                         SIG, bias=bias_t[:, :])
                    nc.sync.dma_start(
                        out_d.ap()[i0:i0 + 128,
                                   a0 + q * 512:a0 + (q + 1) * 512],
                        qt[:, :])

            # Schedule: phase-1 for u cols 0:1024 runs right after warmup
            # (chunk-0 work first); strips 0-7 only need that u range and
            # start while chunks 2-3 still stream in. Tail work sits in the
            # middle; the kernel ends on the final strip's fast drain.
            ph1_ntp0()
            for s in range(4):
                do_strip(s * 128)
            ph1_ntp1()
            ph1_tail()
            tail_block()
            do_half(2048, 2, 0)
            do_half(2048, 2, 1)
            for s in range(4, 15):
                do_strip(s * 128)
            do_half(15 * 128, 128, 0)
            do_final(15 * 128)

    nc.compile()
    return nc


last_results = None


def _host_pack(x, W):
    xT = x.transpose(0, 2, 1).astype(ml_dtypes.bfloat16)  # (B, 320, 2050)
    full = np.zeros((B, 128, KT, L), ml_dtypes.bfloat16)
    full[:, :, 0, :] = xT[:, 0:128]
    full[:, :, 1, :] = xT[:, 128:256]
    full[:, 0:64, 2, :] = xT[:, 256:320]
    xt_main = np.ascontiguousarray(
        full[..., :J_TAIL].reshape(B, 128, KT, 4, CHUNK)
        .transpose(0, 3, 1, 2, 4))
    xt_tail = np.ascontiguousarray(full[..., J_TAIL:L])
    wp = np.zeros((384, 384), ml_dtypes.bfloat16)
    wp[0:320, 0:320] = W.astype(ml_dtypes.bfloat16)
    return xt_main, xt_tail, wp


def kernel(x, W, b, _trace=False):
    global last_results
    x = np.ascontiguousarray(np.asarray(x, dtype=np.float32))
    W = np.asarray(W, dtype=np.float32)
    b = np.asarray(b, dtype=np.float32)
    bias_val = float(b[0])

    if bias_val not in _cache:
        _cache.clear()
        _cache[bias_val] = _build(bias_val)
    nc = _cache[bias_val]

    xt_main, xt_tail, wp = _host_pack(x, W)
    in_maps = [{"xt_main": xt_main[c], "xt_tail": xt_tail[c], "w": wp}
               for c in range(N_CORES)]
    res = run_bass_kernel_spmd(nc, in_maps, core_ids=list(range(N_CORES)),
                               trace=_trace)
    last_results = res
    out = np.empty((B, L, L), dtype=np.float32)
    for c in range(N_CORES):
        out[c, :, :J_TAIL] = res.results[c]["out"].astype(np.float32)
        out[c, :, J_TAIL:] = res.results[c]["out_tail_t"].T.astype(np.float32)
    return out


# revision 7
# speedup vs baseline: 1.0231x; 1.0231x over previous
"""ESM2 contact predictor head on 8 Trainium2 NeuronCores.

Computes out[b, i, j] = sigmoid(x[b,i] @ W @ x[b,j] + bias) for
x: (8, 2050, 320) f32, W: (320, 320) f32, bias: (1,) f32.

Sharding: data-parallel over batch — core c handles batch element c.

Per-core algorithm (all matmuls in bf16 with f32 PSUM accumulation;
bf16 streams at the full PE rate and halves DMA bytes, weight-load
time and SBUF footprint vs f32r; total quantization error ~5e-3 rel,
inside the 2e-2 gate):
  host:  xt = bf16(x[c]).T as 3 K-slabs of 128 partitions (D=320
         zero-padded to 384), pre-chunked so input DMAs are contiguous;
         wp = bf16(W) zero-padded to (384, 384).
  chip:  warmup matmuls release the PE clock-gate while inputs stream in;
         u = wp.T @ xt                      == (x[c] @ W).T, cast to bf16
         (phase 1 runs its chunk-0 halves first so the PE never waits on
         the chunk-1 input DMA);
         out[i, j] = sigmoid(sum_e u[e,i] * xt[e,j] + bias), produced as
         128-row strips: per 1024-col half, 6 matmuls (k-outer, shared
         stationary operand, alternating PSUM banks) -> fused sigmoid+bias
         on ScalarE reading PSUM into a [128, 2048] bf16 strip tile; one
         contiguous 512KB DMA per strip.
         The 2 tail columns (j=2048:2050) are computed transposed
         (2 partitions x 2050) into a separate DRAM tensor; the host
         transposes them into place. Tail work runs mid-schedule; the
         last strip drains as a half plus two 512-col quarters whose
         first sigmoid overlaps the remaining matmuls.
"""

import numpy as np
import ml_dtypes

import concourse.mybir as mybir
import concourse.tile as tile
from concourse import bacc
from concourse.bass_utils import run_bass_kernel_spmd

N_CORES = 8
B, L, D = 8, 2050, 320
KT = 3            # K slabs: 128, 128, 64(zero-padded)
F32 = mybir.dt.float32
BF16 = mybir.dt.bfloat16
SIG = mybir.ActivationFunctionType.Sigmoid

J_TAIL = 2048
CHUNK = 512       # input DMA chunk (columns)

_cache = {}


def _build(bias_val: float):
    nc = bacc.Bacc("TRN2", target_bir_lowering=False, debug=False,
                   num_devices=N_CORES)
    xt_main_d = nc.dram_tensor("xt_main", [4, 128, KT, CHUNK], BF16,
                               kind="ExternalInput")
    xt_tail_d = nc.dram_tensor("xt_tail", [128, KT, 2], BF16,
                               kind="ExternalInput")
    w_d = nc.dram_tensor("w", [384, 384], BF16, kind="ExternalInput")
    out_d = nc.dram_tensor("out", [L, J_TAIL], BF16, kind="ExternalOutput")
    outt_d = nc.dram_tensor("out_tail_t", [2, L], BF16, kind="ExternalOutput")

    w_r = w_d.ap().rearrange("(k p) e -> p k e", p=128)     # (128, 3, 384)

    with tile.TileContext(nc) as tc:
        with (
            tc.tile_pool(name="persist", bufs=1) as pp,
            tc.tile_pool(name="outp", bufs=6) as outp,
            tc.tile_pool(name="psum", bufs=2, space="PSUM") as psp,
        ):
            bias_t = pp.tile([128, 1], F32)
            nc.vector.memset(bias_t[:], bias_val)

            w_sb = pp.tile([128, KT, 384], BF16)
            xt_sb = pp.tile([128, KT, L], BF16)
            nc.sync.dma_start(w_sb[:], w_r[:])
            nc.sync.dma_start(xt_sb[:, :, 0:CHUNK], xt_main_d.ap()[0])
            nc.sync.dma_start(xt_sb[:, :, CHUNK:2 * CHUNK], xt_main_d.ap()[1])
            nc.sync.dma_start(xt_sb[:, :, J_TAIL:L], xt_tail_d.ap())
            nc.sync.dma_start(xt_sb[:, :, 2 * CHUNK:3 * CHUNK], xt_main_d.ap()[2])
            nc.sync.dma_start(xt_sb[:, :, 3 * CHUNK:4 * CHUNK], xt_main_d.ap()[3])

            u_sb = pp.tile([128, KT, L], BF16)

            # PE warmup: dummy matmuls during the input-DMA window so the
            # HAM clock-gate is released before real work starts; the ramp
            # needs ~3us of continuous execution.
            warm_sb = pp.tile([128, 256], F32)
            nc.vector.memset(warm_sb[:], 1.0)
            warm16 = warm_sb.bitcast(BF16)           # (128, 512)
            psw = psp.tile([128, 1024], F32, tag="pair", bufs=3, name="psw")
            for wi in range(11):
                h = (wi % 2) * 512
                nc.tensor.matmul(psw[:, h:h + 512], lhsT=warm16[:, :128],
                                 rhs=warm16[:], start=True, stop=True)
            # preload the sigmoid activation table while DMAs run
            act_warm = pp.tile([128, 1], F32)
            nc.scalar.activation(act_warm[:], bias_t[:], SIG)

            def mm_acc(ps, lhs, j0, nsz):
                """One accumulation group over the 3 K slabs (slab 2
                zero-padded to K=128)."""
                for k in range(KT):
                    nc.tensor.matmul(ps, lhsT=lhs(k),
                                     rhs=xt_sb[:, k, j0:j0 + nsz],
                                     start=(k == 0), stop=(k == KT - 1))

            # ---- phase 1: u = wp.T @ xt  (u[e, i], e on partitions) ----
            def wlhs_of(et):
                def wlhs(k, e0=et * 128):
                    return w_sb[:, k, e0:e0 + 128]
                return wlhs

            def ph1_ntp0():
                # chunk-0 halves of all three e-strips first, so the PE has
                # ~2us of work that doesn't depend on the chunk-1 DMA.
                tiles = []
                for et in range(KT):
                    ps1 = psp.tile([128, 1024], F32, tag="pair", bufs=3,
                                   name="ps1")
                    mm_acc(ps1[:, 0:512], wlhs_of(et), 0, 512)
                    tiles.append(ps1)
                for et in range(KT):
                    ps1 = tiles[et]
                    mm_acc(ps1[:, 512:1024], wlhs_of(et), 512, 512)
                    nc.vector.tensor_copy(u_sb[:, et, 0:1024], ps1[:, :])

            def ph1_ntp1():
                n0 = 1024
                for et in range(KT):
                    ps1 = psp.tile([128, 1024], F32, tag="pair", bufs=3,
                                   name="ps1")
                    mm_acc(ps1[:, 0:512], wlhs_of(et), n0, 512)
                    mm_acc(ps1[:, 512:1024], wlhs_of(et), n0 + 512, 512)
                    nc.vector.tensor_copy(u_sb[:, et, n0:n0 + 1024], ps1[:, :])

            def ph1_tail():
                # u[:, 2048:2050] for the three e-strips in one psum tile
                # (three 2-col accumulation groups at offsets 0/2/4).
                psT = psp.tile([128, 512], F32, tag="small", bufs=2,
                               name="psT")
                for et in range(KT):
                    mm_acc(psT[:, 2 * et:2 * et + 2], wlhs_of(et), J_TAIL, 2)
                for et in range(KT):
                    nc.vector.tensor_copy(u_sb[:, et, J_TAIL:L],
                                          psT[:, 2 * et:2 * et + 2])

            def tail_block():
                # out[:, 2048:2050] for all i, computed transposed (j on
                # partitions), written to its own DRAM tensor; host
                # transposes it into place. Two pair psum tiles hold the
                # four 512-col accumulation groups.
                outT = outp.tile([2, L], BF16, tag="ttail", bufs=1, name="outT")
                for half in range(2):
                    psc = psp.tile([128, 1024], F32, tag="pair", bufs=3,
                                   name="psc")
                    for q in range(2):
                        c0 = (2 * half + q) * 512
                        for k in range(KT):
                            nc.tensor.matmul(psc[:2, q * 512:(q + 1) * 512],
                                             lhsT=xt_sb[:, k, J_TAIL:L],
                                             rhs=u_sb[:, k, c0:c0 + 512],
                                             start=(k == 0), stop=(k == KT - 1))
                    for q in range(2):
                        c0 = (2 * half + q) * 512
                        nc.scalar.activation(outT[:, c0:c0 + 512],
                                             psc[:2, q * 512:(q + 1) * 512],
                                             SIG, bias=bias_t[:2, :])
                psc2 = psp.tile([128, 512], F32, tag="small", bufs=2,
                                name="psc2")
                for k in range(KT):
                    nc.tensor.matmul(psc2[:2, :2], lhsT=xt_sb[:, k, J_TAIL:L],
                                     rhs=u_sb[:, k, J_TAIL:L],
                                     start=(k == 0), stop=(k == KT - 1))
                nc.scalar.activation(outT[:, J_TAIL:L], psc2[:2, :2], SIG,
                                     bias=bias_t[:2, :])
                nc.sync.dma_start(outt_d.ap()[:], outT[:])

            def half_mms(ps, i0, isz, jp):
                a0 = jp * 1024
                for k in range(KT):
                    u_k = u_sb[:, k, i0:i0 + isz]
                    nc.tensor.matmul(ps[:isz, 0:512], lhsT=u_k,
                                     rhs=xt_sb[:, k, a0:a0 + 512],
                                     start=(k == 0), stop=(k == KT - 1))
                    nc.tensor.matmul(ps[:isz, 512:1024], lhsT=u_k,
                                     rhs=xt_sb[:, k, a0 + 512:a0 + 1024],
                                     start=(k == 0), stop=(k == KT - 1))

            def do_strip(i0):
                # full 2048-col strip: two halves -> two sigmoids into one
                # [128, 2048] tile -> one contiguous 512KB DMA.
                big = outp.tile([128, 2048], BF16, tag="strip", bufs=6,
                                name="big")
                for jp in range(2):
                    ps = psp.tile([128, 1024], F32, tag="pair", bufs=3,
                                  name="ps")
                    half_mms(ps, i0, 128, jp)
                    nc.scalar.activation(big[:, jp * 1024:(jp + 1) * 1024],
                                         ps[:, :], SIG, bias=bias_t[:, :])
                nc.sync.dma_start(out_d.ap()[i0:i0 + 128, :], big[:, :])

            def do_half(i0, isz, jp):
                # one 1024-col half with its own DMA (i-tail, final strip)
                a0 = jp * 1024
                ps = psp.tile([128, 1024], F32, tag="pair", bufs=3, name="ps")
                half_mms(ps, i0, isz, jp)
                half = outp.tile([128, 1024], BF16, tag="half", bufs=2,
                                 name="half")
                nc.scalar.activation(half[:isz, :], ps[:isz, :], SIG,
                                     bias=bias_t[:isz, :])
                nc.sync.dma_start(out_d.ap()[i0:i0 + isz, a0:a0 + 1024],
                                  half[:isz, :])

            def do_final(i0):
                # last half (jp=1): quarter-grouped matmuls so the first
                # sigmoid overlaps the remaining three matmuls; per-quarter
                # DMA for a fast drain.
                a0 = 1024
                ps = psp.tile([128, 1024], F32, tag="pair", bufs=3, name="ps")
                for q in range(2):
                    for k in range(KT):
                        nc.tensor.matmul(
                            ps[:, q * 512:(q + 1) * 512],
                            lhsT=u_sb[:, k, i0:i0 + 128],
                            rhs=xt_sb[:, k, a0 + q * 512:a0 + (q + 1) * 512],
                            start=(k == 0), stop=(k == KT - 1))
                for q in range(2):
                    qt = outp.tile([128, 512], BF16, tag="fin", bufs=2,
                                   name="fin")
                    nc.scalar.activation(qt[:, :],
                                         ps[:, q * 512:(q + 1) * 512],
                                         SIG, bias=bias_t[:, :])
                    nc.sync.dma_start(
                        out_d.ap()[i0:i0 + 128,
                                   a0 + q * 512:a0 + (q + 1) * 512],
                        qt[:, :])

            # Schedule: phase-1 for u cols 0:1024 runs right after warmup
            # (chunk-0 work first); strips 0-7 only need that u range and
            # start while chunks 2-3 still stream in. Tail work sits in the
            # middle; the kernel ends on the final strip's fast drain.
            ph1_ntp0()
            ph1_tail()
            do_half(2048, 2, 0)
            do_half(2048, 2, 1)
            ph1_ntp1()
            tail_block()
            for s in range(15):
                do_strip(s * 128)
            do_half(15 * 128, 128, 0)
            do_final(15 * 128)

    nc.compile()
    return nc


last_results = None


def _host_pack(x, W):
    xT = x.transpose(0, 2, 1).astype(ml_dtypes.bfloat16)  # (B, 320, 2050)
    full = np.zeros((B, 128, KT, L), ml_dtypes.bfloat16)
    full[:, :, 0, :] = xT[:, 0:128]
    full[:, :, 1, :] = xT[:, 128:256]
    full[:, 0:64, 2, :] = xT[:, 256:320]
    xt_main = np.ascontiguousarray(
        full[..., :J_TAIL].reshape(B, 128, KT, 4, CHUNK)
        .transpose(0, 3, 1, 2, 4))
    xt_tail = np.ascontiguousarray(full[..., J_TAIL:L])
    wp = np.zeros((384, 384), ml_dtypes.bfloat16)
    wp[0:320, 0:320] = W.astype(ml_dtypes.bfloat16)
    return xt_main, xt_tail, wp


def kernel(x, W, b, _trace=False):
    global last_results
    x = np.ascontiguousarray(np.asarray(x, dtype=np.float32))
    W = np.asarray(W, dtype=np.float32)
    b = np.asarray(b, dtype=np.float32)
    bias_val = float(b[0])

    if bias_val not in _cache:
        _cache.clear()
        _cache[bias_val] = _build(bias_val)
    nc = _cache[bias_val]

    xt_main, xt_tail, wp = _host_pack(x, W)
    in_maps = [{"xt_main": xt_main[c], "xt_tail": xt_tail[c], "w": wp}
               for c in range(N_CORES)]
    res = run_bass_kernel_spmd(nc, in_maps, core_ids=list(range(N_CORES)),
                               trace=_trace)
    last_results = res
    out = np.empty((B, L, L), dtype=np.float32)
    for c in range(N_CORES):
        out[c, :, :J_TAIL] = res.results[c]["out"].astype(np.float32)
        out[c, :, J_TAIL:] = res.results[c]["out_tail_t"].T.astype(np.float32)
    return out


# revision 8
# speedup vs baseline: 1.0513x; 1.0275x over previous
"""ESM2 contact predictor head on 8 Trainium2 NeuronCores.

Computes out[b, i, j] = sigmoid(x[b,i] @ W @ x[b,j] + bias) for
x: (8, 2050, 320) f32, W: (320, 320) f32, bias: (1,) f32.

Sharding: data-parallel over batch - core c handles batch element c.

Per-core algorithm (all matmuls in bf16 with f32 PSUM accumulation; bf16
streams at the full PE rate - unlike fp16, which measures ~12% slower on
real TRN2 - and halves DMA bytes vs f32r; quantization error ~5e-3 rel,
inside the 2e-2 gate):
  host:  xt = bf16(x[c]).T as 3 K-slabs of 128 partitions (D=320
         zero-padded to 384), pre-chunked so input DMAs are contiguous;
         wp = bf16(W) zero-padded to (384, 384).
  chip:  11 warmup matmuls ramp the PE clock while inputs stream in
         (first chunk is usable only ~12.5us in due to DMA latency);
         u = wp.T @ xt == (x[c] @ W).T, cast to bf16; phase 1 runs its
         chunk-0 halves first so the PE never waits on the chunk-1 DMA;
         out[i, j] = sigmoid(sum_e u[e,i] * xt[e,j] + bias) as 128-row x
         1024-col half-strips: 6 matmuls (k-outer, shared stationary,
         alternating PSUM banks) -> fused sigmoid+bias on ScalarE -> bf16
         DMA. Strips 0-7 jp=0 run on chunks 0-1 while chunks 2-3 arrive.
         The 2 tail columns (j=2048:2050) are computed transposed into a
         separate DRAM tensor (host transposes them into place); all tail
         work sits mid-schedule. The final half drains as two 512-col
         quarters whose first sigmoid overlaps the remaining matmuls.
"""

import numpy as np
import ml_dtypes

import concourse.mybir as mybir
import concourse.tile as tile
from concourse import bacc
from concourse.bass_utils import run_bass_kernel_spmd

N_CORES = 8
B, L, D = 8, 2050, 320
KT = 3            # K slabs: 128, 128, 64(zero-padded)
F32 = mybir.dt.float32
BF16 = mybir.dt.bfloat16
SIG = mybir.ActivationFunctionType.Sigmoid

J_TAIL = 2048
CHUNK = 512       # input DMA chunk (columns)

_cache = {}


def _build(bias_val: float):
    nc = bacc.Bacc("TRN2", target_bir_lowering=False, debug=False,
                   num_devices=N_CORES)
    xt_main_d = nc.dram_tensor("xt_main", [4, 128, KT, CHUNK], BF16,
                               kind="ExternalInput")
    xt_tail_d = nc.dram_tensor("xt_tail", [128, KT, 2], BF16,
                               kind="ExternalInput")
    w_d = nc.dram_tensor("w", [384, 384], BF16, kind="ExternalInput")
    out_d = nc.dram_tensor("out", [L, J_TAIL], BF16, kind="ExternalOutput")
    outt_d = nc.dram_tensor("out_tail_t", [2, L], BF16, kind="ExternalOutput")

    w_r = w_d.ap().rearrange("(k p) e -> p k e", p=128)     # (128, 3, 384)

    with tile.TileContext(nc) as tc:
        with (
            tc.tile_pool(name="persist", bufs=1) as pp,
            tc.tile_pool(name="outp", bufs=8) as outp,
            tc.tile_pool(name="psum", bufs=2, space="PSUM") as psp,
        ):
            bias_t = pp.tile([128, 1], F32)
            nc.vector.memset(bias_t[:], bias_val)

            w_sb = pp.tile([128, KT, 384], BF16)
            xt_sb = pp.tile([128, KT, L], BF16)
            nc.sync.dma_start(w_sb[:], w_r[:])
            nc.sync.dma_start(xt_sb[:, :, 0:CHUNK], xt_main_d.ap()[0])
            nc.sync.dma_start(xt_sb[:, :, CHUNK:2 * CHUNK], xt_main_d.ap()[1])
            nc.sync.dma_start(xt_sb[:, :, J_TAIL:L], xt_tail_d.ap())
            nc.sync.dma_start(xt_sb[:, :, 2 * CHUNK:3 * CHUNK], xt_main_d.ap()[2])
            nc.sync.dma_start(xt_sb[:, :, 3 * CHUNK:4 * CHUNK], xt_main_d.ap()[3])

            u_sb = pp.tile([128, KT, L], BF16)

            warm_sb = pp.tile([128, 256], F32)
            nc.vector.memset(warm_sb[:], 1.0)
            warm16 = warm_sb.bitcast(BF16)           # (128, 512)
            psw = psp.tile([128, 1024], F32, tag="pair", bufs=3, name="psw")
            for wi in range(11):
                h = (wi % 2) * 512
                nc.tensor.matmul(psw[:, h:h + 512], lhsT=warm16[:, :128],
                                 rhs=warm16[:], start=True, stop=True)
            act_warm = pp.tile([128, 1], F32)
            nc.scalar.activation(act_warm[:], bias_t[:], SIG)

            def mm_acc(ps, lhs, j0, nsz):
                for k in range(KT):
                    nc.tensor.matmul(ps, lhsT=lhs(k),
                                     rhs=xt_sb[:, k, j0:j0 + nsz],
                                     start=(k == 0), stop=(k == KT - 1))

            def wlhs_of(et):
                def wlhs(k, e0=et * 128):
                    return w_sb[:, k, e0:e0 + 128]
                return wlhs

            def ph1_ntp0():
                # chunk-0 halves of all three e-strips first, so the PE has
                # ~2us of work that doesn't depend on the chunk-1 input DMA.
                tiles = []
                for et in range(KT):
                    ps1 = psp.tile([128, 1024], F32, tag="pair", bufs=3,
                                   name="ps1")
                    mm_acc(ps1[:, 0:512], wlhs_of(et), 0, 512)
                    tiles.append(ps1)
                for et in range(KT):
                    ps1 = tiles[et]
                    mm_acc(ps1[:, 512:1024], wlhs_of(et), 512, 512)
                    nc.vector.tensor_copy(u_sb[:, et, 0:1024], ps1[:, :])

            def ph1_ntp1():
                n0 = 1024
                for et in range(KT):
                    ps1 = psp.tile([128, 1024], F32, tag="pair", bufs=3,
                                   name="ps1")
                    mm_acc(ps1[:, 0:512], wlhs_of(et), n0, 512)
                    mm_acc(ps1[:, 512:1024], wlhs_of(et), n0 + 512, 512)
                    nc.vector.tensor_copy(u_sb[:, et, n0:n0 + 1024], ps1[:, :])

            def ph1_tail():
                # u[:, 2048:2050] for the three e-strips in one psum tile
                # (three 2-col accumulation groups at offsets 0/2/4).
                psT = psp.tile([128, 512], F32, tag="small", bufs=2,
                               name="psT")
                for et in range(KT):
                    mm_acc(psT[:, 2 * et:2 * et + 2], wlhs_of(et), J_TAIL, 2)
                for et in range(KT):
                    nc.vector.tensor_copy(u_sb[:, et, J_TAIL:L],
                                          psT[:, 2 * et:2 * et + 2])

            def tail_block():
                outT = outp.tile([2, L], BF16, tag="ttail", bufs=1, name="outT")
                for half in range(2):
                    psc = psp.tile([128, 1024], F32, tag="pair", bufs=3,
                                   name="psc")
                    for q in range(2):
                        c0 = (2 * half + q) * 512
                        for k in range(KT):
                            nc.tensor.matmul(psc[:2, q * 512:(q + 1) * 512],
                                             lhsT=xt_sb[:, k, J_TAIL:L],
                                             rhs=u_sb[:, k, c0:c0 + 512],
                                             start=(k == 0), stop=(k == KT - 1))
                    for q in range(2):
                        c0 = (2 * half + q) * 512
                        nc.scalar.activation(outT[:, c0:c0 + 512],
                                             psc[:2, q * 512:(q + 1) * 512],
                                             SIG, bias=bias_t[:2, :])
                psc2 = psp.tile([128, 512], F32, tag="small", bufs=2,
                                name="psc2")
                for k in range(KT):
                    nc.tensor.matmul(psc2[:2, :2], lhsT=xt_sb[:, k, J_TAIL:L],
                                     rhs=u_sb[:, k, J_TAIL:L],
                                     start=(k == 0), stop=(k == KT - 1))
                nc.scalar.activation(outT[:, J_TAIL:L], psc2[:2, :2], SIG,
                                     bias=bias_t[:2, :])
                nc.sync.dma_start(outt_d.ap()[:], outT[:])

            def do_half(i0, isz, jp, fine=False):
                a0 = jp * 1024
                ps = psp.tile([128, 1024], F32, tag="pair", bufs=3, name="ps")
                if fine:
                    # quarter-grouped: group 0 closes after 3 matmuls so its
                    # sigmoid overlaps group 1's matmuls (fast drain).
                    for q in range(2):
                        for k in range(KT):
                            nc.tensor.matmul(
                                ps[:isz, q * 512:(q + 1) * 512],
                                lhsT=u_sb[:, k, i0:i0 + isz],
                                rhs=xt_sb[:, k, a0 + q * 512:a0 + (q + 1) * 512],
                                start=(k == 0), stop=(k == KT - 1))
                else:
                    for k in range(KT):
                        u_k = u_sb[:, k, i0:i0 + isz]
                        nc.tensor.matmul(ps[:isz, 0:512], lhsT=u_k,
                                         rhs=xt_sb[:, k, a0:a0 + 512],
                                         start=(k == 0), stop=(k == KT - 1))
                        nc.tensor.matmul(ps[:isz, 512:1024], lhsT=u_k,
                                         rhs=xt_sb[:, k, a0 + 512:a0 + 1024],
                                         start=(k == 0), stop=(k == KT - 1))
                if fine:
                    for q in range(2):  # acts overlap: groups closed early
                        qt = outp.tile([128, 512], BF16, tag="fin", bufs=2,
                                       name="fin")
                        nc.scalar.activation(qt[:isz, :],
                                             ps[:isz, q * 512:(q + 1) * 512],
                                             SIG, bias=bias_t[:isz, :])
                        nc.sync.dma_start(
                            out_d.ap()[i0:i0 + isz,
                                       a0 + q * 512:a0 + (q + 1) * 512],
                            qt[:isz, :])
                else:
                    half = outp.tile([128, 1024], BF16, tag="strip", bufs=8,
                                     name="half")
                    nc.scalar.activation(half[:isz, :], ps[:isz, :], SIG,
                                         bias=bias_t[:isz, :])
                    nc.sync.dma_start(out_d.ap()[i0:i0 + isz, a0:a0 + 1024],
                                      half[:isz, :])

            ph1_ntp0()
            for s in range(8):
                do_half(s * 128, 128, 0)
            ph1_ntp1()
            ph1_tail()
            tail_block()
            do_half(2048, 2, 0)
            do_half(2048, 2, 1)
            for s in range(8):
                do_half(s * 128, 128, 1)
            for s in range(8, 15):
                do_half(s * 128, 128, 0)
                do_half(s * 128, 128, 1)
            do_half(15 * 128, 128, 0)
            do_half(15 * 128, 128, 1, fine=True)

    nc.compile()
    return nc


last_results = None


def _host_pack(x, W):
    xT = x.transpose(0, 2, 1).astype(ml_dtypes.bfloat16)  # (B, 320, 2050)
    full = np.zeros((B, 128, KT, L), ml_dtypes.bfloat16)
    full[:, :, 0, :] = xT[:, 0:128]
    full[:, :, 1, :] = xT[:, 128:256]
    full[:, 0:64, 2, :] = xT[:, 256:320]
    xt_main = np.ascontiguousarray(
        full[..., :J_TAIL].reshape(B, 128, KT, 4, CHUNK)
        .transpose(0, 3, 1, 2, 4))
    xt_tail = np.ascontiguousarray(full[..., J_TAIL:L])
    wp = np.zeros((384, 384), ml_dtypes.bfloat16)
    wp[0:320, 0:320] = W.astype(ml_dtypes.bfloat16)
    return xt_main, xt_tail, wp


def kernel(x, W, b, _trace=False):
    global last_results
    x = np.ascontiguousarray(np.asarray(x, dtype=np.float32))
    W = np.asarray(W, dtype=np.float32)
    b = np.asarray(b, dtype=np.float32)
    bias_val = float(b[0])

    if bias_val not in _cache:
        _cache.clear()
        _cache[bias_val] = _build(bias_val)
    nc = _cache[bias_val]

    xt_main, xt_tail, wp = _host_pack(x, W)
    in_maps = [{"xt_main": xt_main[c], "xt_tail": xt_tail[c], "w": wp}
               for c in range(N_CORES)]
    res = run_bass_kernel_spmd(nc, in_maps, core_ids=list(range(N_CORES)),
                               trace=_trace)
    last_results = res
    out = np.empty((B, L, L), dtype=np.float32)
    for c in range(N_CORES):
        out[c, :, :J_TAIL] = res.results[c]["out"].astype(np.float32)
        out[c, :, J_TAIL:] = res.results[c]["out_tail_t"].T.astype(np.float32)
    return out
